# revision 20
# baseline (speedup 1.0000x reference)
"""Single-head attention on 8 TRN2 NeuronCores, data-parallel over batch.

Per core (one batch element b):
  x_b [2048, 768] f32 -> Q = x Wq, K = x Wk, V = x Wv (head 64)
  scores^T[k, q] = K^T-slice.T @ Q^T / 8 ; E = exp(scores);
  U = [V,1]^T-weighted sums of E give out + denominator.

v5 layout/schedule:
  - All x tiles stream via SWDGE cast-DMA f32->bf16. Weights and the
    precomputed identity/duplication matrices go first on the (small)
    HWDGE queue so nothing gates the projections. A fence read after
    tile 5 defers tiles 8-15 so they don't steal HBM bandwidth from the
    kt-loop's critical set (strips 0-1 + weights).
  - x^T is built by per-tile PE transpose units (6 chunks -> one PSUM
    tile -> one evac) so work flows tile-by-tile as DMAs land. Strip 1's
    projections run in half-strip pieces to start earlier.
  - Projections pack A=[Wq|Wk] and B=[Wv|Wq]; K^T is duplicated into
    both partition halves with one PE matmul against [I64|I64] so score
    matmuls run as concurrent 2-way row-tiled pairs.
  - exp on ScalarE, one [128, 1024] ACTIVATE per k-tile from PSUM; the
    kt loop is software-pipelined in emission order (scores(kt) ->
    exp(kt) -> PV(kt-1) -> fillers) so exp never queues behind PV or
    phase-2 work on the PE FIFO. Strips 2-3 and s1's K-dup/V build run
    as fillers inside qh0's PE slack; qh0's output tiles inside qh1's.
  - PV uses lhsT = [V, ones] (M=65); psum row 64 is the softmax
    denominator. U transposes run in bf16; outputs collect in one SBUF
    buffer per q-half and leave as a single DMA each.
"""

import numpy as np

import concourse.bass as bass
import concourse.tile as tile
from concourse import bacc, mybir
from concourse.bass_utils import run_bass_kernel_spmd

B, S, D, H = 8, 2048, 768, 64
P = 128
NT = S // P  # 16 seq tiles
NCH = D // P  # 6 emb chunks
QC = 512
N_CORES = 8
F32 = mybir.dt.float32
BF16 = mybir.dt.bfloat16
EXP = mybir.ActivationFunctionType.Exp
SCALE = float(1.0 / np.sqrt(H))


def build_kernel():
    nc = bacc.Bacc("TRN2", num_devices=N_CORES)
    x_ext = nc.declare_dram_parameter("x", [S, D], F32, isOutput=False)
    wk_ext = nc.declare_dram_parameter("Wk", [D, H], F32, isOutput=False)
    wq_ext = nc.declare_dram_parameter("Wq", [D, H], F32, isOutput=False)
    wv_ext = nc.declare_dram_parameter("Wv", [D, H], F32, isOutput=False)
    idb_ext = nc.declare_dram_parameter("idb", [P, 2, P], BF16,
                                        isOutput=False)
    out_ext = nc.declare_dram_parameter("out", [S, H], F32, isOutput=True)

    with tile.TileContext(nc) as tc:
        _body(nc, tc, x_ext, wq_ext, wk_ext, wv_ext, idb_ext, out_ext)
    nc.compile()
    return nc


def _body(nc, tc, x_ext, wq_ext, wk_ext, wv_ext, idb_ext, out_ext):
    with (
        tc.tile_pool(name="singles", bufs=1) as singles,
        tc.tile_pool(name="xn", bufs=3) as xn_pool,
        tc.tile_pool(name="et", bufs=3) as et_pool,
        tc.tile_pool(name="fin", bufs=4) as fin_pool,
        tc.tile_pool(name="ph2", bufs=2, space="PSUM") as ph2,
        tc.tile_pool(name="ss", bufs=2, space="PSUM") as ss_pool,
        tc.tile_pool(name="uu", bufs=2, space="PSUM") as u_pool,
    ):
        # ---- HWDGE (sync): identities + weights, before anything big
        idb = singles.tile([P, 2, P], BF16, tag="idb")
        nc.sync.dma_start(out=idb, in_=idb_ext[:, :, :])
        ident_bf = idb[:, 0, :]
        dupI = idb[:, 1, :]  # [I64|I64] in partitions 64-127

        wq_st = singles.tile([P, NCH, H], F32, tag="wst_q")
        wk_st = singles.tile([P, NCH, H], F32, tag="wst_k")
        wv_st = singles.tile([P, NCH, H], F32, tag="wst_v")
        for w_st, w_ext in ((wq_st, wq_ext), (wk_st, wk_ext), (wv_st, wv_ext)):
            nc.sync.dma_start(
                out=w_st, in_=w_ext.rearrange("(c p) h -> p c h", p=P))

        # ---- SWDGE (gpsimd): x cast-DMAs f32->bf16; tiles 8-15 wait
        # behind a fence read of tile 5 so the critical set (strips 0-1
        # + weights) gets full HBM bandwidth first.
        xn_tiles = [xn_pool.tile([P, D], BF16, name=f"xn_{st}",
                                 tag=f"xn_{st}", bufs=1)
                    for st in range(NT)]

        def dma_x(st):
            nc.gpsimd.dma_start(
                out=xn_tiles[st], in_=x_ext[st * P:(st + 1) * P, :])

        for st in range(8):
            dma_x(st)
        # tiles 8-15: a tiny guard write makes each DMA data-dependent on
        # tile st-3, so the scheduler cannot hoist them above the critical
        # set (strips 0-1) and they stream in order, ~3 in flight. The
        # guards live on the gpsimd queue where waiting blocks nothing.
        for st in range(8, NT):
            nc.gpsimd.tensor_copy(xn_tiles[st][0:1, 0:4],
                                  xn_tiles[st - 3][0:1, 0:4])
            dma_x(st)

        # warm the exp table set while DMAs stream
        dummy = singles.tile([P, 8], BF16, tag="dummy")
        nc.scalar.activation(dummy, idb[:, 0, 0:8], EXP, scale=SCALE)

        # weight packs A=[Wq|Wk], B=[Wv|Wq] (DVE)
        wA = singles.tile([P, NCH, P], BF16, tag="wA")
        wB = singles.tile([P, NCH, P], BF16, tag="wB")
        nc.vector.tensor_copy(wA[:, :, 0:H], wq_st)
        nc.vector.tensor_copy(wA[:, :, H:P], wk_st)
        nc.vector.tensor_copy(wB[:, :, 0:H], wv_st)
        nc.vector.tensor_copy(wB[:, :, H:P], wq_st)

        # ---- persistent SBUF state
        xt_sb = singles.tile([P, NCH, NT, P], BF16, tag="xt_sb")  # x^T
        qkt = singles.tile([P, S], BF16, tag="qkt")   # [Q^T; K^T]
        qvt = singles.tile([P, S], BF16, tag="qvt")   # [V^T; Q^T]
        ktd = singles.tile([P, S], BF16, tag="ktd")   # K^T both halves
        vp = singles.tile([P, NT, H + 1], BF16, tag="vp")  # V' = [V, 1]
        nc.vector.memset(vp[:, :, H:H + 1], 1.0)

        # ---- phase-2 units. During the pre-loop the score pool is idle,
        # so units alternate between the two PSUM pools for a 4-deep
        # rotation (2 slots would serialize on evac pacing).
        pools = {"state": 0, "free": False}

        def _p2pool():
            if not pools["free"]:
                return ph2
            pools["state"] ^= 1
            return ph2 if pools["state"] else ss_pool

        def emit_trans(st):
            pst = _p2pool().tile([P, NCH, P], BF16, tag="ss", name="pst")
            for c in range(NCH):
                nc.tensor.transpose(
                    pst[:, c, :], xn_tiles[st][:, c * P:(c + 1) * P],
                    ident_bf)
            if st < 8 and st % 2 == 1:
                nc.scalar.copy(out=xt_sb[:, :, st, :], in_=pst)
            else:
                nc.vector.tensor_copy(xt_sb[:, :, st, :], pst)

        def emit_proj(w_t, dst, sc, half=None):
            t0 = sc * 4 if half in (None, 0) else sc * 4 + 2
            nt = 4 if half is None else 2
            tsl = slice(t0, t0 + nt)
            sl = slice(t0 * P, (t0 + nt) * P)
            ps = _p2pool().tile([P, QC], F32, tag="ss", name="ps")
            for c in range(NCH):
                nc.tensor.matmul(ps[:, 0:nt * P], w_t[:, c, :],
                                 xt_sb[:, c, tsl, :],
                                 start=(c == 0), stop=(c == NCH - 1))
            nc.vector.tensor_copy(dst[:, sl], ps[:, 0:nt * P])

        def emit_projA(sc, half=None):
            emit_proj(wA, qkt, sc, half)

        def emit_projB(sc, half=None):
            emit_proj(wB, qvt, sc, half)

        def emit_kdup(sc):
            sl = slice(sc * QC, (sc + 1) * QC)
            psK = _p2pool().tile([P, QC], F32, tag="ss", name="psK")
            nc.tensor.matmul(psK, dupI[64:P, :], qkt[64:P, sl],
                             start=True, stop=True)
            nc.vector.tensor_copy(ktd[:, sl], psK)

        def emit_vtrans(sc, half=None):
            tiles = range(sc * 4, (sc + 1) * 4) if half is None else (
                range(sc * 4, sc * 4 + 2) if half == 0 else
                range(sc * 4 + 2, (sc + 1) * 4))
            n = len(tiles)
            psv = _p2pool().tile([P, 4, H], BF16, tag="ss", name="psv")
            for i, t in enumerate(tiles):
                nc.tensor.transpose(
                    psv[:, i, :], qvt[0:H, t * P:(t + 1) * P],
                    ident_bf[:H, :H])
            t0 = tiles[0]
            nc.vector.tensor_copy(vp[:, t0:t0 + n, 0:H], psv[:, 0:n, :])

        # ---- pre-loop: strip 0 fully, strip 1 except kdup/vtrans
        pools["free"] = True
        for st in range(4):
            emit_trans(st)
        emit_projA(0)
        emit_projB(0)
        emit_trans(4)
        emit_trans(5)
        emit_projA(1, 0)
        emit_projB(1, 0)
        emit_trans(6)
        emit_trans(7)
        emit_kdup(0)
        emit_vtrans(0)
        emit_projA(1, 1)
        emit_projB(1, 1)
        pools["free"] = False

        # ---- output tail for one 128-row q tile; batched DMA per q-half
        ut_tiles = {}
        ob_tiles = {}

        def emit_out(qt):
            # one unit handles the pair (qt, qt+1): 2 transposes share one
            # PSUM tile and one [128, 2] reciprocal
            ut = ut_tiles[qt // 4]
            ob = ob_tiles[qt // 8]
            pso = ph2.tile([P, 2, H + 2], BF16, tag="ss", name="pso")
            for i in range(2):
                nc.tensor.transpose(
                    pso[:, i, 0:H + 1],
                    ut[:, (qt % 4 + i) * P:(qt % 4 + i + 1) * P],
                    ident_bf[:H + 1, :H + 1])
            rcp = fin_pool.tile([P, 2], F32, tag="rcp", name="rcp")
            nc.vector.reciprocal(rcp, pso[:, :, H])
            for i in range(2):
                nc.vector.tensor_scalar_mul(
                    ob[:, qt % 8 + i, :], pso[:, i, 0:H], rcp[:, i:i + 1])
            if qt % 8 == 6:
                half = qt // 8
                nc.sync.dma_start(
                    out=out_ext[half * 1024:(half + 1) * 1024, :].rearrange(
                        "(t p) h -> p t h", p=P),
                    in_=ob)

        # filler at key (qh, k) is emitted in pipeline group k+1; sc(kt)
        # precedes fill(kt-1) on the PE FIFO, so a unit feeding sc(kt)/PV(kt)
        # must sit at key <= kt-2 / kt-1 respectively.
        filler = {
            (0, 1): [("kd", 1, None), ("tr", 8, None)],
            (0, 2): [("vt", 1, None), ("tr", 9, None)],
            (0, 3): [("tr", 10, None), ("tr", 11, None)],
            (0, 4): [("A", 2, None)],
            (0, 5): [("kd", 2, None)],
            (0, 6): [("B", 2, None)],
            (0, 7): [("vt", 2, None), ("tr", 12, None)],
            (0, 8): [("tr", 13, None), ("tr", 14, None)],
            (0, 9): [("tr", 15, None), ("A", 3, None)],
            (0, 10): [("kd", 3, None)],
            (0, 11): [("B", 3, None), ("vt", 3, 0)],
            (0, 12): [("vt", 3, 1)],
            (1, 0): [("out", 0, None)],
            (1, 2): [("out", 2, None)],
            (1, 4): [("out", 4, None)],
            (1, 6): [("out", 6, None)],
        }

        def run_filler(qh, kt):
            for kind, a1, a2 in filler.get((qh, kt), []):
                if kind == "tr":
                    emit_trans(a1)
                elif kind == "A":
                    emit_projA(a1, a2)
                elif kind == "B":
                    emit_projB(a1, a2)
                elif kind == "kd":
                    emit_kdup(a1)
                elif kind == "vt":
                    emit_vtrans(a1, a2)
                elif kind == "out":
                    emit_out(a1)

        # ---- main attention loops: per q-half, 16 k-tiles,
        # software-pipelined emission: sc(kt), exp(kt), PV(kt-1), fill(kt-1)
        for qh in range(2):
            q0 = qh * 1024
            U = [u_pool.tile([H + 1, QC], F32, tag="pu",
                             name=f"U{qh}_{j}") for j in range(2)]
            et_tiles = {}
            for kt in range(NT):
                ksl = slice(kt * P, (kt + 1) * P)
                ss = ss_pool.tile([P, 2, QC], F32, tag="ss", name="ss")
                nc.tensor.matmul(
                    ss[:, 0, :], ktd[0:H, ksl],
                    qkt[0:H, q0:q0 + QC], start=True, stop=True)
                nc.tensor.matmul(
                    ss[:, 1, :], ktd[H:P, ksl],
                    qvt[H:P, q0 + QC:q0 + 2 * QC], start=True, stop=True)
                et = et_pool.tile([P, 2, QC], BF16, name="et")
                nc.scalar.activation(et, ss, EXP, scale=SCALE)
                et_tiles[kt] = et
                if kt > 0:
                    for j in range(2):
                        nc.tensor.matmul(
                            U[j], vp[:, kt - 1, :], et_tiles[kt - 1][:, j, :],
                            start=(kt - 1 == 0), stop=False)
                    del et_tiles[kt - 1]
                    run_filler(qh, kt - 1)

            # final PV per j, evacuating U[j] (bf16) as soon as it closes
            ob_tiles[qh] = fin_pool.tile([P, 8, H], F32, tag=f"ob{qh}",
                                         name=f"ob{qh}", bufs=1)
            for j in range(2):
                nc.tensor.matmul(
                    U[j], vp[:, NT - 1, :], et_tiles[NT - 1][:, j, :],
                    start=False, stop=True)
                ut = fin_pool.tile([H + 1, QC], BF16, tag=f"ut{j}",
                                   name=f"ut{qh}_{j}")
                nc.vector.tensor_copy(ut, U[j])
                ut_tiles[qh * 2 + j] = ut
            run_filler(qh, NT - 1)

        # qh1's own output tiles (qh0's ran as qh1 fillers)
        for qt in range(8, 16, 2):
            emit_out(qt)


_cached_nc = None


def _make_idb():
    idb = np.zeros((P, 2, P), dtype=np.float32)
    idb[:, 0, :] = np.eye(P)
    i64 = np.eye(64)
    idb[64:P, 1, 0:64] = i64
    idb[64:P, 1, 64:P] = i64
    try:
        import ml_dtypes
        return idb.astype(ml_dtypes.bfloat16)
    except ImportError:
        # 0.0/1.0 are exact in bf16: truncate the f32 bit pattern
        return (idb.view(np.uint32) >> 16).astype(np.uint16)


def kernel(**inputs):
    global _cached_nc
    x = np.ascontiguousarray(inputs["x"], dtype=np.float32)
    wk = np.ascontiguousarray(inputs["Wk"], dtype=np.float32)
    wq = np.ascontiguousarray(inputs["Wq"], dtype=np.float32)
    wv = np.ascontiguousarray(inputs["Wv"], dtype=np.float32)
    assert x.shape == (B, S, D)

    if _cached_nc is None:
        _cached_nc = build_kernel()
    nc = _cached_nc

    idb = _make_idb()
    in_maps = [{"x": x[b], "Wk": wk, "Wq": wq, "Wv": wv, "idb": idb}
               for b in range(B)]
    res = run_bass_kernel_spmd(nc, in_maps, core_ids=list(range(N_CORES)))
    return np.stack([res.results[i]["out"] for i in range(N_CORES)], axis=0)


# revision 21
# speedup vs baseline: 1.1293x; 1.1293x over previous
"""Single-head attention on 8 TRN2 NeuronCores, data-parallel over batch.

Per core (one batch element b):
  x_b [2048, 768] f32 -> Q = x Wq, K = x Wk, V = x Wv (head 64)
  scores^T[k, q] = K^T-slice.T @ Q^T / 8 ; E = exp(scores);
  U = [V,1]^T-weighted sums of E give out + denominator.

v3 layout/schedule:
  - x arrives via SWDGE cast-DMA f32->bf16, issued at the very head of
    the gpsimd queue so HBM streaming starts immediately.
  - Projections pack A=[Wq|Wk] and B=[Wv|Wq]; K^T is duplicated into both
    partition halves with one PE matmul against [I64|I64] so the score
    matmuls run as concurrent 2-way row-tiled pairs.
  - exp on ScalarE, one [128, 1024] ACTIVATE per k-tile straight from
    PSUM. The kt loop is software-pipelined in emission order
    (scores(kt) -> exp(kt) -> PV(kt-1) -> fillers) so exp never queues
    behind PV or phase-2 work on the PE FIFO.
  - Separate PSUM pools: scores double-buffer (4 banks) + phase-2 (2) +
    PV accumulators (2) = exactly 8 banks.
  - PV uses lhsT = [V, ones] (M=65); psum row 64 is the softmax
    denominator. Outputs collect in one SBUF buffer per q-half and leave
    as a single DMA each.
"""

import numpy as np

import concourse.bass as bass
import concourse.tile as tile
from concourse import bacc, mybir
from concourse.bass_utils import run_bass_kernel_spmd
from concourse.masks import make_identity

B, S, D, H = 8, 2048, 768, 64
P = 128
NT = S // P  # 16 seq tiles
NCH = D // P  # 6 emb chunks
QC = 512
N_CORES = 8
F32 = mybir.dt.float32
BF16 = mybir.dt.bfloat16
EXP = mybir.ActivationFunctionType.Exp
SCALE = float(1.0 / np.sqrt(H))


def build_kernel():
    nc = bacc.Bacc("TRN2", num_devices=N_CORES)
    x_ext = nc.declare_dram_parameter("x", [S, D], F32, isOutput=False)
    wk_ext = nc.declare_dram_parameter("Wk", [D, H], F32, isOutput=False)
    wq_ext = nc.declare_dram_parameter("Wq", [D, H], F32, isOutput=False)
    wv_ext = nc.declare_dram_parameter("Wv", [D, H], F32, isOutput=False)
    out_ext = nc.declare_dram_parameter("out", [S, H], F32, isOutput=True)

    with tile.TileContext(nc) as tc:
        _body(nc, tc, x_ext, wq_ext, wk_ext, wv_ext, out_ext)
    nc.compile()
    return nc


def _body(nc, tc, x_ext, wq_ext, wk_ext, wv_ext, out_ext):
    with (
        tc.tile_pool(name="singles", bufs=1) as singles,
        tc.tile_pool(name="xn", bufs=3) as xn_pool,
        tc.tile_pool(name="et", bufs=3) as et_pool,
        tc.tile_pool(name="fin", bufs=4) as fin_pool,
        tc.tile_pool(name="ph2", bufs=2, space="PSUM") as ph2,
        tc.tile_pool(name="ss", bufs=2, space="PSUM") as ss_pool,
        tc.tile_pool(name="uu", bufs=2, space="PSUM") as u_pool,
    ):
        # ---- x cast-DMAs head the gpsimd queue; identities slot in
        # after strip 0+2 tiles so the first transposes aren't blocked.
        xn_tiles = [xn_pool.tile([P, D], BF16, name=f"xn_{st}",
                                 tag=f"xn_{st}", bufs=1)
                    for st in range(NT)]

        def dma_x(st):
            nc.gpsimd.dma_start(
                out=xn_tiles[st], in_=x_ext[st * P:(st + 1) * P, :])

        for st in range(6):
            dma_x(st)

        ident = singles.tile([P, P], F32)
        make_identity(nc, ident)
        ident_bf = singles.tile([P, P], BF16, tag="ident_bf")
        make_identity(nc, ident_bf)
        # [I64|I64] in partitions 64-127: K^T-duplication stationary.
        dupI = singles.tile([P, P], BF16, tag="dupI")
        nc.vector.tensor_copy(dupI[64:P, 0:64], ident_bf[64:P, 64:P])
        nc.vector.tensor_copy(dupI[64:P, 64:P], ident_bf[64:P, 64:P])
        vp = singles.tile([P, NT, H + 1], BF16, tag="vp")  # V' = [V, 1]
        nc.vector.memset(vp[:, :, H:H + 1], 1.0)

        # warm the exp table set while everything else is still loading
        dummy = singles.tile([P, 8], BF16, tag="dummy")
        nc.scalar.activation(dummy, ident_bf[:, 0:8], EXP, scale=SCALE)

        for st in range(6, NT):
            dma_x(st)

        # ---- weights: DMA f32, pack A=[Wq|Wk], B=[Wv|Wq] in bf16
        wq_st = singles.tile([P, NCH, H], F32, tag="wst_q")
        wk_st = singles.tile([P, NCH, H], F32, tag="wst_k")
        wv_st = singles.tile([P, NCH, H], F32, tag="wst_v")
        for w_st, w_ext in ((wq_st, wq_ext), (wk_st, wk_ext), (wv_st, wv_ext)):
            nc.sync.dma_start(
                out=w_st, in_=w_ext.rearrange("(c p) h -> p c h", p=P))
        wA = singles.tile([P, NCH, P], BF16, tag="wA")
        wB = singles.tile([P, NCH, P], BF16, tag="wB")
        nc.vector.tensor_copy(wA[:, :, 0:H], wq_st)
        nc.vector.tensor_copy(wA[:, :, H:P], wk_st)
        nc.vector.tensor_copy(wB[:, :, 0:H], wv_st)
        nc.vector.tensor_copy(wB[:, :, H:P], wq_st)

        # ---- persistent SBUF state
        xt_sb = singles.tile([P, NCH, NT, P], BF16, tag="xt_sb")  # x^T
        qkt = singles.tile([P, S], BF16, tag="qkt")   # [Q^T; K^T]
        qvt = singles.tile([P, S], BF16, tag="qvt")   # [V^T; Q^T]
        ktd = singles.tile([P, S], BF16, tag="ktd")   # K^T both halves

        # ---- phase-2 units (per strip sc of 4 seq tiles)
        def emit_trans(sc, c):
            tsl = slice(sc * 4, (sc + 1) * 4)
            pst = ph2.tile([P, 4, P], BF16, tag="ph2", name="pst")
            for t in range(4):
                nc.tensor.transpose(
                    pst[:, t, :],
                    xn_tiles[sc * 4 + t][:, c * P:(c + 1) * P],
                    ident_bf)
            # strips 0-1 evac on the (still idle) scalar engine
            if sc < 2:
                nc.scalar.copy(out=xt_sb[:, c, tsl, :], in_=pst)
            else:
                nc.vector.tensor_copy(xt_sb[:, c, tsl, :], pst)

        def emit_projA(sc):
            sl = slice(sc * QC, (sc + 1) * QC)
            tsl = slice(sc * 4, (sc + 1) * 4)
            psA = ph2.tile([P, QC], F32, tag="ph2", name="psA")
            for c in range(NCH):
                nc.tensor.matmul(psA, wA[:, c, :], xt_sb[:, c, tsl, :],
                                 start=(c == 0), stop=(c == NCH - 1))
            nc.vector.tensor_copy(qkt[:, sl], psA)

        def emit_projB(sc):
            sl = slice(sc * QC, (sc + 1) * QC)
            tsl = slice(sc * 4, (sc + 1) * 4)
            psB = ph2.tile([P, QC], F32, tag="ph2", name="psB")
            for c in range(NCH):
                nc.tensor.matmul(psB, wB[:, c, :], xt_sb[:, c, tsl, :],
                                 start=(c == 0), stop=(c == NCH - 1))
            nc.vector.tensor_copy(qvt[:, sl], psB)

        def emit_kdup(sc):
            sl = slice(sc * QC, (sc + 1) * QC)
            psK = ph2.tile([P, QC], F32, tag="ph2", name="psK")
            nc.tensor.matmul(psK, dupI[64:P, :], qkt[64:P, sl],
                             start=True, stop=True)
            nc.vector.tensor_copy(ktd[:, sl], psK)

        def emit_vtrans(sc):
            psv = ph2.tile([P, 4, H], BF16, tag="ph2", name="psv")
            for t in range(4):
                off = sc * QC + t * P
                nc.tensor.transpose(
                    psv[:, t, :], qvt[0:H, off:off + P], ident_bf[:H, :H])
            nc.vector.tensor_copy(vp[:, sc * 4:(sc + 1) * 4, 0:H], psv)

        def emit_strip(sc):
            for c in range(NCH):
                emit_trans(sc, c)
            emit_projA(sc)
            emit_projB(sc)
            emit_kdup(sc)
            emit_vtrans(sc)

        emit_strip(0)
        emit_strip(1)

        # ---- output tail for one 128-row q tile; batched DMA per q-half
        ut_tiles = {}
        ob_tiles = {}

        def emit_out(qt):
            ut = ut_tiles[qt // 4]
            ob = ob_tiles[qt // 8]
            pso = ph2.tile([P, H + 1], F32, tag="ph2", name="pso")
            nc.tensor.transpose(
                pso, ut[:, (qt % 4) * P:(qt % 4 + 1) * P],
                ident[:H + 1, :H + 1])
            rcp = fin_pool.tile([P, 1], F32, tag="rcp", name="rcp")
            nc.vector.reciprocal(rcp, pso[:, H:H + 1])
            nc.vector.tensor_scalar_mul(ob[:, qt % 8, :], pso[:, 0:H], rcp)
            if qt % 8 == 7:
                half = qt // 8
                nc.sync.dma_start(
                    out=out_ext[half * 1024:(half + 1) * 1024, :].rearrange(
                        "(t p) h -> p t h", p=P),
                    in_=ob)

        # filler at key (qh, k) is emitted in pipeline group k+1; sc(kt)
        # precedes fill(kt-1) on the PE FIFO, so a unit feeding sc(kt)/PV(kt)
        # must sit at key <= kt-2 / kt-1 respectively.
        filler = {
            (0, 1): [("tr", 2, 0), ("tr", 2, 1)],
            (0, 2): [("tr", 2, 2), ("tr", 2, 3)],
            (0, 3): [("tr", 2, 4), ("tr", 2, 5)],
            (0, 4): [("A", 2, 0)],
            (0, 5): [("kd", 2, 0), ("B", 2, 0), ("tr", 3, 0)],
            (0, 6): [("vt", 2, 0), ("tr", 3, 1)],
            (0, 7): [("tr", 3, 2), ("tr", 3, 3)],
            (0, 8): [("tr", 3, 4), ("tr", 3, 5)],
            (0, 9): [("A", 3, 0)],
            (0, 10): [("kd", 3, 0), ("B", 3, 0)],
            (0, 11): [("vt", 3, 0)],
            (1, 0): [("out", 0, 0)],
            (1, 1): [("out", 1, 0)],
            (1, 2): [("out", 2, 0)],
            (1, 3): [("out", 3, 0)],
            (1, 4): [("out", 4, 0)],
            (1, 5): [("out", 5, 0)],
            (1, 6): [("out", 6, 0)],
            (1, 7): [("out", 7, 0)],
        }

        def run_filler(qh, kt):
            for kind, a1, a2 in filler.get((qh, kt), []):
                if kind == "tr":
                    emit_trans(a1, a2)
                elif kind == "A":
                    emit_projA(a1)
                elif kind == "B":
                    emit_projB(a1)
                elif kind == "kd":
                    emit_kdup(a1)
                elif kind == "vt":
                    emit_vtrans(a1)
                elif kind == "out":
                    emit_out(a1)

        # ---- main attention loops: per q-half, 16 k-tiles,
        # software-pipelined emission: sc(kt), exp(kt), PV(kt-1), fill(kt-1)
        for qh in range(2):
            q0 = qh * 1024
            U = [u_pool.tile([H + 1, QC], F32, tag="pu",
                             name=f"U{qh}_{j}") for j in range(2)]
            et_tiles = {}
            for kt in range(NT):
                ksl = slice(kt * P, (kt + 1) * P)
                ss = ss_pool.tile([P, 2, QC], F32, tag="ss", name="ss")
                nc.tensor.matmul(
                    ss[:, 0, :], ktd[0:H, ksl],
                    qkt[0:H, q0:q0 + QC], start=True, stop=True)
                nc.tensor.matmul(
                    ss[:, 1, :], ktd[H:P, ksl],
                    qvt[H:P, q0 + QC:q0 + 2 * QC], start=True, stop=True)
                et = et_pool.tile([P, 2, QC], BF16, name="et")
                nc.scalar.activation(et, ss, EXP, scale=SCALE)
                et_tiles[kt] = et
                if kt > 0:
                    for j in range(2):
                        nc.tensor.matmul(
                            U[j], vp[:, kt - 1, :], et_tiles[kt - 1][:, j, :],
                            start=(kt - 1 == 0), stop=False)
                    del et_tiles[kt - 1]
                    run_filler(qh, kt - 1)
            for j in range(2):
                nc.tensor.matmul(
                    U[j], vp[:, NT - 1, :], et_tiles[NT - 1][:, j, :],
                    start=False, stop=True)
            run_filler(qh, NT - 1)

            # evacuate U and stage this half's output buffer
            ob_tiles[qh] = fin_pool.tile([P, 8, H], F32, tag=f"ob{qh}",
                                         name=f"ob{qh}", bufs=1)
            for j in range(2):
                ut = fin_pool.tile([H + 1, QC], F32, tag=f"ut{j}",
                                   name=f"ut{qh}_{j}")
                nc.vector.tensor_copy(ut, U[j])
                ut_tiles[qh * 2 + j] = ut

        # qh1's own output tiles (qh0's ran as qh1 fillers)
        for qt in range(8, 16):
            emit_out(qt)


_cached_nc = None


def kernel(**inputs):
    global _cached_nc
    x = np.ascontiguousarray(inputs["x"], dtype=np.float32)
    wk = np.ascontiguousarray(inputs["Wk"], dtype=np.float32)
    wq = np.ascontiguousarray(inputs["Wq"], dtype=np.float32)
    wv = np.ascontiguousarray(inputs["Wv"], dtype=np.float32)
    assert x.shape == (B, S, D)

    if _cached_nc is None:
        _cached_nc = build_kernel()
    nc = _cached_nc

    in_maps = [{"x": x[b], "Wk": wk, "Wq": wq, "Wv": wv} for b in range(B)]
    res = run_bass_kernel_spmd(nc, in_maps, core_ids=list(range(N_CORES)))
    return np.stack([res.results[i]["out"] for i in range(N_CORES)], axis=0)
